# revision 19
# baseline (speedup 1.0000x reference)
"""FAGCN message-passing kernel for 8 Trainium2 NeuronCores.

Strategy (edge-parallel via dst-ownership, v4 — tableless):
  - Nodes are assigned to the 8 cores snake-wise in degree-sorted order, so
    every core owns ~N/8 nodes, ~E/8 edges, and sees the same degree profile
    (the compiled SPMD program is shared; only the index inputs differ).
  - Gate decomposition: tanh(Linear([h_dst, h_src])) = tanh(p1[dst] + p2[src] + b)
    with p1 = x @ w_dst, p2 = x @ w_src.
  - Phase-2 indirect gathers read raw f32 x rows STRAIGHT from the input
    tensor (no device-built table, so gathers start immediately); p2 is
    computed per-edge on the DVE, which hides entirely under the gather
    span (the [P,1] indirect-DMA issue rate on GpSimd is the wall).
  - norm[src] is a pure function of in-degrees (index data), so the host
    ships it pre-expanded in slot layout (normsl), like idx.
  - dst-side (own) scalars come from a host-sharded copy of the owned rows
    (xown, in tile order) -> p1b_own, norm_own per tile; this small DVE
    block overlaps the first gather batch.
  - Per 128-node tile (dst-major, degree-sorted so slot padding is tiny):
    gate is ACT tanh with per-partition bias (p1b_own);
    z[dst] = norm[dst] * sum_s tanh(p1b[dst] + p2[src]) * norm[src] * x[src].
"""

import os
import sys

sys.path.insert(0, "/opt/trn_rl_repo")

import numpy as np

P = 128
D64 = 64

LAST_RESULTS = None  # BassKernelResults of the most recent HW run (for profiling)


def _ceil_to(a, m):
    return ((a + m - 1) // m) * m


class Plan:
    pass


def _prep(x, gate_w, gate_b, src, dst, ncores=8):
    """Host-side sharding: shapes/constants + per-core input maps."""
    x = np.asarray(x, dtype=np.float32)
    gate_w = np.asarray(gate_w, dtype=np.float32)
    gate_b = np.asarray(gate_b, dtype=np.float32)
    src = np.asarray(src).astype(np.int64)
    dst = np.asarray(dst).astype(np.int64)

    N, D = x.shape
    assert D == 64
    E = src.shape[0]

    pl = Plan()
    pl.N, pl.D, pl.E, pl.ncores = N, D, E, ncores
    pl.NPAD = _ceil_to(N + 1, P)
    # sentinel row: x = 0, norm slot = 0 -> zero contribution
    pl.SENT = pl.NPAD - 1

    deg = np.bincount(dst, minlength=N).astype(np.int64)

    # snake assignment over degree-sorted nodes -> per-core node lists
    order = np.argsort(-deg, kind="stable")
    n8 = _ceil_to(N, ncores)
    order_p = np.concatenate([order, np.full(n8 - N, -1, dtype=np.int64)])
    blocks = order_p.reshape(-1, ncores).copy()
    blocks[1::2] = blocks[1::2, ::-1]
    core_nodes = np.ascontiguousarray(blocks.T)  # [ncores, npc]
    npc = core_nodes.shape[1]
    pl.NPC_PAD = _ceil_to(npc, P)
    pl.TILES = pl.NPC_PAD // P
    pad = np.full((ncores, pl.NPC_PAD - npc), -1, dtype=np.int64)
    core_nodes = np.concatenate([core_nodes, pad], axis=1)  # [ncores, NPC_PAD]
    pl.core_nodes = core_nodes

    node_deg = np.where(core_nodes >= 0, deg[np.clip(core_nodes, 0, N - 1)], 0)
    deg_tiles = node_deg.reshape(ncores, pl.TILES, P)
    Kt = deg_tiles.max(axis=(0, 2)).astype(np.int64)
    Kt = np.maximum(Kt, 1)
    pl.Kt = Kt
    pl.SX = int(Kt.sum())

    # CSR by dst
    e_order = np.argsort(dst, kind="stable")
    src_sorted = src[e_order]
    ends = np.cumsum(deg)
    starts = ends - deg

    # shared inputs
    xp = np.zeros((pl.NPAD, D), dtype=np.float32)
    xp[:N] = x
    wrep = np.empty((P, 128), dtype=np.float32)
    wrep[:, 0:64] = gate_w[0, 64:128][None, :]   # w_src
    wrep[:, 64:128] = gate_w[0, 0:64][None, :]   # w_dst
    b128 = np.full((P, 1), float(np.asarray(gate_b).reshape(-1)[0]), dtype=np.float32)
    normv = (1.0 / np.sqrt(np.maximum(deg, 1))).astype(np.float32)  # [N]

    in_maps = []
    karange = np.arange(int(Kt.max()))[None, :]
    for c in range(ncores):
        # one slot-column stream per tile: [slot1..slotK]; idx = x row of the
        # source node, normsl = norm[source] (0 on sentinel slots)
        idx = np.full((P, pl.SX), pl.SENT, dtype=np.int32)
        normsl = np.zeros((P, pl.SX), dtype=np.float32)
        koff = 0
        for t in range(pl.TILES):
            K = int(Kt[t])
            nodes = core_nodes[c, t * P : (t + 1) * P]  # [128]
            real = nodes >= 0
            d = np.where(real, deg[np.clip(nodes, 0, N - 1)], 0)
            st = np.where(real, starts[np.clip(nodes, 0, N - 1)], 0)
            mask = karange[:, :K] < d[:, None]  # [128, K]
            pos = st[:, None] + karange[:, :K]
            vals = src_sorted[np.minimum(pos, E - 1)]
            idx[:, koff : koff + K] = np.where(mask, vals, pl.SENT).astype(np.int32)
            normsl[:, koff : koff + K] = np.where(mask, normv[vals], 0.0).astype(
                np.float32
            )
            koff += K
        # host-sharded owned rows, partition-major [P, TILES*64] (contiguous load)
        nodes_c = core_nodes[c]
        xown = np.zeros((pl.NPC_PAD, D), dtype=np.float32)
        realc = nodes_c >= 0
        xown[realc] = x[nodes_c[realc]]
        xown = np.ascontiguousarray(
            xown.reshape(pl.TILES, P, D).transpose(1, 0, 2).reshape(P, pl.TILES * D)
        )
        dgow_flat = np.where(realc, deg[np.clip(nodes_c, 0, N - 1)], 0).astype(
            np.float32
        )
        dgow = np.ascontiguousarray(dgow_flat.reshape(pl.TILES, P).T)  # [P, TILES]
        in_maps.append(
            {
                "xp": xp,
                "wrep": wrep,
                "b128": b128,
                "idx": idx,
                "normsl": normsl,
                "xown": xown,
                "dgow": dgow,
            }
        )
    return pl, in_maps


def _build_nc(pl):
    """Build the shared SPMD Bass/Tile program."""
    import concourse.bass as bass
    import concourse.bacc as bacc
    import concourse.mybir as mybir
    import concourse.tile as tile

    f32 = mybir.dt.float32
    i32 = mybir.dt.int32
    AF = mybir.ActivationFunctionType
    OP = mybir.AluOpType

    D = pl.D
    TILES = pl.TILES
    Kt = [int(k) for k in pl.Kt]
    SX = pl.SX

    nc = bacc.Bacc("TRN2", target_bir_lowering=False, debug=False, num_devices=pl.ncores)
    xp_d = nc.dram_tensor("xp", [pl.NPAD, D], f32, kind="ExternalInput")
    wrep_d = nc.dram_tensor("wrep", [P, 128], f32, kind="ExternalInput")
    b128_d = nc.dram_tensor("b128", [P, 1], f32, kind="ExternalInput")
    idx_d = nc.dram_tensor("idx", [P, SX], i32, kind="ExternalInput")
    normsl_d = nc.dram_tensor("normsl", [P, SX], f32, kind="ExternalInput")
    xown_d = nc.dram_tensor("xown", [P, TILES * D], f32, kind="ExternalInput")
    dgow_d = nc.dram_tensor("dgow", [P, TILES], f32, kind="ExternalInput")
    z_d = nc.dram_tensor("z", [pl.NPC_PAD, D], f32, kind="ExternalOutput")
    bf16 = mybir.dt.bfloat16
    xbt_d = nc.dram_tensor("xbt", [pl.NPAD, D], bf16)  # per-chunk writes
    xbf_d = nc.dram_tensor("xbf", [pl.NPAD, D], bf16)  # consolidated copy

    # batched phase-2 gathers: group tiles while sum(K) <= BATCH_K
    BATCH_K = 64
    batches = []
    b0 = 0
    while b0 < TILES:
        b1 = b0 + 1
        ks = Kt[b0]
        while b1 < TILES and ks + Kt[b1] <= BATCH_K:
            ks += Kt[b1]
            b1 += 1
        batches.append((b0, b1, ks))
        b0 = b1

    with tile.TileContext(nc) as tc:
        with (
            tc.tile_pool(name="consts", bufs=1) as cpool,
            tc.tile_pool(name="own", bufs=3) as p1pool,
            tc.tile_pool(name="gather", bufs=2) as gpool,
            tc.tile_pool(name="work", bufs=2) as wpool,
        ):
            # gathers depend only on idx — load it first so they start at once
            idx_sb = cpool.tile([P, SX], i32)
            nc.sync.dma_start(out=idx_sb[:], in_=idx_d[:, :])
            normsl_sb = cpool.tile([P, SX], f32)
            nc.sync.dma_start(out=normsl_sb[:], in_=normsl_d[:, :])
            wrep_sb = cpool.tile([P, 128], f32)
            nc.sync.dma_start(out=wrep_sb[:], in_=wrep_d[:, :])
            b128_sb = cpool.tile([P, 1], f32)
            nc.sync.dma_start(out=b128_sb[:], in_=b128_d[:, :])

            # ---- own-node scalars (p1b, norm) per tile; overlaps first gathers ----
            p1bT = cpool.tile([P, TILES], f32)
            normow = cpool.tile([P, TILES], f32)
            dgow_sb = cpool.tile([P, TILES], f32)
            nc.sync.dma_start(out=dgow_sb[:], in_=dgow_d[:, :])
            dgclip = cpool.tile([P, TILES], f32)
            nc.vector.tensor_scalar(
                out=dgclip[:], in0=dgow_sb[:], scalar1=1.0, scalar2=None, op0=OP.max
            )
            dgrec = cpool.tile([P, TILES], f32)
            nc.vector.reciprocal(out=dgrec[:], in_=dgclip[:])
            nc.scalar.activation(out=normow[:], in_=dgrec[:], func=AF.Sqrt)

            OB = 8
            for t0 in range(0, TILES, OB):
                tn = min(OB, TILES - t0)
                xo = p1pool.tile([P, OB * 64], f32, tag="xo")
                xov = xo[:].rearrange("p (i f) -> p i f", f=64)
                nc.sync.dma_start(
                    out=xov[:, 0:tn, :],
                    in_=xown_d[:, t0 * 64 : (t0 + tn) * 64].rearrange(
                        "p (t f) -> p t f", f=64
                    ),
                )
                tmpo = p1pool.tile([P, OB * 64], f32, tag="tmpo")
                tov = tmpo[:].rearrange("p (i f) -> p i f", f=64)
                nc.vector.tensor_tensor(
                    out=tov[:, 0:tn, :],
                    in0=xov[:, 0:tn, :],
                    in1=wrep_sb[:, 64:128]
                    .rearrange("p (o f) -> p o f", o=1)
                    .to_broadcast([P, tn, 64]),
                    op=OP.mult,
                )
                redo = wpool.tile([P, OB], f32, tag="redo")
                nc.vector.tensor_reduce(
                    out=redo[:, 0:tn],
                    in_=tov[:, 0:tn, :],
                    axis=mybir.AxisListType.X,
                    op=OP.add,
                )
                nc.vector.tensor_scalar(
                    out=p1bT[:, t0 : t0 + tn],
                    in0=redo[:, 0:tn],
                    scalar1=b128_sb[:, 0:1],
                    scalar2=None,
                    op0=OP.add,
                )

            # ---- bf16 copy of x (overlaps the first f32 gather batches) ----
            CHB = pl.NPAD // P  # rows per partition (block-major)
            xpv = xp_d[0 : pl.NPAD, :].rearrange("(p c) f -> p c f", p=P)
            xbtv = xbt_d[0 : pl.NPAD, :].rearrange("(p c) f -> p c f", p=P)
            CC = 48
            for c0 in range(0, CHB, CC):
                cn = min(CC, CHB - c0)
                xa = p1pool.tile([P, CC * D64], f32, tag="xa")
                xav = xa[:].rearrange("p (i f) -> p i f", f=D64)
                nc.sync.dma_start(out=xav[:, 0:cn, :], in_=xpv[:, c0 : c0 + cn, :])
                xb = p1pool.tile([P, CC * D64], bf16, tag="xb")
                xbv = xb[:].rearrange("p (i f) -> p i f", f=D64)
                nc.scalar.activation(
                    out=xbv[:, 0:cn, :], in_=xav[:, 0:cn, :], func=AF.Copy
                )
                nc.sync.dma_start(out=xbtv[:, c0 : c0 + cn, :], in_=xbv[:, 0:cn, :])
            # one consolidation copy so bf16 gathers wait on a single DMA sem
            nc.sync.dma_start(out=xbf_d[:, :], in_=xbt_d[:, :])

            # ---- phase 2: gather x rows + per-edge gate + aggregate ----
            coff = [0]
            for k in Kt:
                coff.append(coff[-1] + k)

            FP32_BATCHES = 2  # gathered from f32 x while the bf16 copy builds
            for bi, (b0, b1, ks) in enumerate(batches):
                s0 = coff[b0]
                ga = gpool.tile([P, (BATCH_K + 8) * D64], f32, tag="ga")
                gav = ga[:].rearrange("p (k f) -> p k f", f=D64)
                if bi < FP32_BATCHES:
                    # one [P,1] indirect per slot column (only validated HW shape)
                    for col in range(ks):
                        nc.gpsimd.indirect_dma_start(
                            out=ga[:, col * D64 : (col + 1) * D64],
                            out_offset=None,
                            in_=xp_d[:, :],
                            in_offset=bass.IndirectOffsetOnAxis(
                                ap=idx_sb[:, s0 + col : s0 + col + 1], axis=0
                            ),
                        )
                else:
                    gb = gpool.tile([P, (BATCH_K + 8) * D64], bf16, tag="gb")
                    for col in range(ks):
                        nc.gpsimd.indirect_dma_start(
                            out=gb[:, col * D64 : (col + 1) * D64],
                            out_offset=None,
                            in_=xbf_d[:, :],
                            in_offset=bass.IndirectOffsetOnAxis(
                                ap=idx_sb[:, s0 + col : s0 + col + 1], axis=0
                            ),
                        )
                    nc.scalar.activation(
                        out=ga[:, 0 : ks * D64], in_=gb[:, 0 : ks * D64], func=AF.Copy
                    )
                koff = 0
                for t in range(b0, b1):
                    K = Kt[t]
                    xsl = gav[:, koff : koff + K, :]
                    # p2 = x_src . w_src per edge
                    t2 = wpool.tile([P, K * 64], f32, tag="t2")
                    nc.vector.tensor_tensor(
                        out=t2[:].rearrange("p (k f) -> p k f", f=64),
                        in0=xsl,
                        in1=wrep_sb[:, 0:64]
                        .rearrange("p (o f) -> p o f", o=1)
                        .to_broadcast([P, K, 64]),
                        op=OP.mult,
                    )
                    p2d = wpool.tile([P, K], f32, tag="p2d")
                    nc.vector.tensor_reduce(
                        out=p2d[:],
                        in_=t2[:].rearrange("p (k f) -> p k f", f=64),
                        axis=mybir.AxisListType.X,
                        op=OP.add,
                    )
                    tt = wpool.tile([P, K], f32, tag="tt")
                    nc.scalar.activation(
                        out=tt[:], in_=p2d[:], func=AF.Tanh, bias=p1bT[:, t : t + 1]
                    )
                    ee = wpool.tile([P, K], f32, tag="ee")
                    nc.vector.tensor_tensor(
                        out=ee[:],
                        in0=tt[:],
                        in1=normsl_sb[:, s0 + koff : s0 + koff + K],
                        op=OP.mult,
                    )
                    m = wpool.tile([P, K * 64], f32, tag="m")
                    eev = (
                        ee[:]
                        .rearrange("p (k o) -> p k o", o=1)
                        .to_broadcast([P, K, 64])
                    )
                    nc.vector.tensor_tensor(
                        out=m[:].rearrange("p (k f) -> p k f", f=64),
                        in0=xsl,
                        in1=eev,
                        op=OP.mult,
                    )
                    red = wpool.tile([P, 64], f32, tag="red")
                    nc.vector.tensor_reduce(
                        out=red[:],
                        in_=m[:].rearrange("p (k f) -> p f k", f=64),
                        axis=mybir.AxisListType.X,
                        op=OP.add,
                    )
                    zt = wpool.tile([P, 64], f32, tag="zt")
                    nc.vector.tensor_scalar(
                        out=zt[:],
                        in0=red[:],
                        scalar1=normow[:, t : t + 1],
                        scalar2=None,
                        op0=OP.mult,
                    )
                    nc.sync.dma_start(out=z_d[t * P : (t + 1) * P, :], in_=zt[:])
                    koff += K
    nc.compile()
    return nc


_BUILD_CACHE = {}


def build(x, gate_w, gate_b, src, dst, ncores=8):
    pl, in_maps = _prep(x, gate_w, gate_b, src, dst, ncores)
    key = (pl.N, pl.E, pl.ncores, tuple(int(k) for k in pl.Kt))
    nc = _BUILD_CACHE.get(key)
    if nc is None:
        nc = _build_nc(pl)
        _BUILD_CACHE[key] = nc
    return pl, in_maps, nc


def _assemble(pl, outs):
    N, D = pl.N, pl.D
    z = np.zeros((N, D), dtype=np.float32)
    for c in range(pl.ncores):
        nodes = pl.core_nodes[c]
        real = nodes >= 0
        z[nodes[real]] = outs[c][real]
    return z


def kernel(x, gate_w, gate_b, src, dst):
    global LAST_RESULTS
    from concourse.bass_utils import run_bass_kernel_spmd

    pl, in_maps, nc = build(x, gate_w, gate_b, src, dst)
    res = run_bass_kernel_spmd(
        nc,
        in_maps,
        core_ids=list(range(pl.ncores)),
        trace=bool(int(os.environ.get("FAGCN_TRACE", "0"))),
    )
    LAST_RESULTS = res
    outs = [r["z"] for r in res.results]
    return _assemble(pl, outs)


# revision 20
# speedup vs baseline: 1.0670x; 1.0670x over previous
"""FAGCN message-passing kernel for 8 Trainium2 NeuronCores.

Strategy (edge-parallel via dst-ownership, v4 — tableless):
  - Nodes are assigned to the 8 cores snake-wise in degree-sorted order, so
    every core owns ~N/8 nodes, ~E/8 edges, and sees the same degree profile
    (the compiled SPMD program is shared; only the index inputs differ).
  - Gate decomposition: tanh(Linear([h_dst, h_src])) = tanh(p1[dst] + p2[src] + b)
    with p1 = x @ w_dst, p2 = x @ w_src.
  - Phase-2 indirect gathers read raw f32 x rows STRAIGHT from the input
    tensor (no device-built table, so gathers start immediately); p2 is
    computed per-edge on the DVE, which hides entirely under the gather
    span (the [P,1] indirect-DMA issue rate on GpSimd is the wall).
  - norm[src] is a pure function of in-degrees (index data), so the host
    ships it pre-expanded in slot layout (normsl), like idx.
  - dst-side (own) scalars come from a host-sharded copy of the owned rows
    (xown, in tile order) -> p1b_own, norm_own per tile; this small DVE
    block overlaps the first gather batch.
  - Per 128-node tile (dst-major, degree-sorted so slot padding is tiny):
    gate is ACT tanh with per-partition bias (p1b_own);
    z[dst] = norm[dst] * sum_s tanh(p1b[dst] + p2[src]) * norm[src] * x[src].
"""

import os
import sys

sys.path.insert(0, "/opt/trn_rl_repo")

import numpy as np

P = 128
D64 = 64

LAST_RESULTS = None  # BassKernelResults of the most recent HW run (for profiling)


def _ceil_to(a, m):
    return ((a + m - 1) // m) * m


class Plan:
    pass


def _prep(x, gate_w, gate_b, src, dst, ncores=8):
    """Host-side sharding: shapes/constants + per-core input maps."""
    x = np.asarray(x, dtype=np.float32)
    gate_w = np.asarray(gate_w, dtype=np.float32)
    gate_b = np.asarray(gate_b, dtype=np.float32)
    src = np.asarray(src).astype(np.int64)
    dst = np.asarray(dst).astype(np.int64)

    N, D = x.shape
    assert D == 64
    E = src.shape[0]

    pl = Plan()
    pl.N, pl.D, pl.E, pl.ncores = N, D, E, ncores
    pl.NPAD = _ceil_to(N + 1, P)
    # sentinel row: x = 0, norm slot = 0 -> zero contribution
    pl.SENT = pl.NPAD - 1

    deg = np.bincount(dst, minlength=N).astype(np.int64)

    # snake assignment over degree-sorted nodes -> per-core node lists
    order = np.argsort(-deg, kind="stable")
    n8 = _ceil_to(N, ncores)
    order_p = np.concatenate([order, np.full(n8 - N, -1, dtype=np.int64)])
    blocks = order_p.reshape(-1, ncores).copy()
    blocks[1::2] = blocks[1::2, ::-1]
    core_nodes = np.ascontiguousarray(blocks.T)  # [ncores, npc]
    npc = core_nodes.shape[1]
    pl.NPC_PAD = _ceil_to(npc, P)
    pl.TILES = pl.NPC_PAD // P
    pad = np.full((ncores, pl.NPC_PAD - npc), -1, dtype=np.int64)
    core_nodes = np.concatenate([core_nodes, pad], axis=1)  # [ncores, NPC_PAD]
    pl.core_nodes = core_nodes

    node_deg = np.where(core_nodes >= 0, deg[np.clip(core_nodes, 0, N - 1)], 0)
    deg_tiles = node_deg.reshape(ncores, pl.TILES, P)
    Kt = deg_tiles.max(axis=(0, 2)).astype(np.int64)
    Kt = np.maximum(Kt, 1)
    pl.Kt = Kt
    pl.SX = int(Kt.sum())

    # CSR by dst
    e_order = np.argsort(dst, kind="stable")
    src_sorted = src[e_order]
    ends = np.cumsum(deg)
    starts = ends - deg

    # shared inputs
    xp = np.zeros((pl.NPAD, D), dtype=np.float32)
    xp[:N] = x
    wrep = np.empty((P, 128), dtype=np.float32)
    wrep[:, 0:64] = gate_w[0, 64:128][None, :]   # w_src
    wrep[:, 64:128] = gate_w[0, 0:64][None, :]   # w_dst
    b128 = np.full((P, 1), float(np.asarray(gate_b).reshape(-1)[0]), dtype=np.float32)
    normv = (1.0 / np.sqrt(np.maximum(deg, 1))).astype(np.float32)  # [N]

    in_maps = []
    karange = np.arange(int(Kt.max()))[None, :]
    for c in range(ncores):
        # one slot-column stream per tile: [slot1..slotK]; idx = x row of the
        # source node, normsl = norm[source] (0 on sentinel slots)
        idx = np.full((P, pl.SX), pl.SENT, dtype=np.int32)
        normsl = np.zeros((P, pl.SX), dtype=np.float32)
        koff = 0
        for t in range(pl.TILES):
            K = int(Kt[t])
            nodes = core_nodes[c, t * P : (t + 1) * P]  # [128]
            real = nodes >= 0
            d = np.where(real, deg[np.clip(nodes, 0, N - 1)], 0)
            st = np.where(real, starts[np.clip(nodes, 0, N - 1)], 0)
            mask = karange[:, :K] < d[:, None]  # [128, K]
            pos = st[:, None] + karange[:, :K]
            vals = src_sorted[np.minimum(pos, E - 1)]
            idx[:, koff : koff + K] = np.where(mask, vals, pl.SENT).astype(np.int32)
            normsl[:, koff : koff + K] = np.where(mask, normv[vals], 0.0).astype(
                np.float32
            )
            koff += K
        # host-sharded owned rows, partition-major [P, TILES*64] (contiguous load)
        nodes_c = core_nodes[c]
        xown = np.zeros((pl.NPC_PAD, D), dtype=np.float32)
        realc = nodes_c >= 0
        xown[realc] = x[nodes_c[realc]]
        xown = np.ascontiguousarray(
            xown.reshape(pl.TILES, P, D).transpose(1, 0, 2).reshape(P, pl.TILES * D)
        )
        dgow_flat = np.where(realc, deg[np.clip(nodes_c, 0, N - 1)], 0).astype(
            np.float32
        )
        dgow = np.ascontiguousarray(dgow_flat.reshape(pl.TILES, P).T)  # [P, TILES]
        in_maps.append(
            {
                "xp": xp,
                "wrep": wrep,
                "b128": b128,
                "idx": idx,
                "normsl": normsl,
                "xown": xown,
                "dgow": dgow,
            }
        )
    return pl, in_maps


def _build_nc(pl):
    """Build the shared SPMD Bass/Tile program."""
    import concourse.bass as bass
    import concourse.bacc as bacc
    import concourse.mybir as mybir
    import concourse.tile as tile

    f32 = mybir.dt.float32
    i32 = mybir.dt.int32
    AF = mybir.ActivationFunctionType
    OP = mybir.AluOpType

    D = pl.D
    TILES = pl.TILES
    Kt = [int(k) for k in pl.Kt]
    SX = pl.SX

    nc = bacc.Bacc("TRN2", target_bir_lowering=False, debug=False, num_devices=pl.ncores)
    xp_d = nc.dram_tensor("xp", [pl.NPAD, D], f32, kind="ExternalInput")
    wrep_d = nc.dram_tensor("wrep", [P, 128], f32, kind="ExternalInput")
    b128_d = nc.dram_tensor("b128", [P, 1], f32, kind="ExternalInput")
    idx_d = nc.dram_tensor("idx", [P, SX], i32, kind="ExternalInput")
    normsl_d = nc.dram_tensor("normsl", [P, SX], f32, kind="ExternalInput")
    xown_d = nc.dram_tensor("xown", [P, TILES * D], f32, kind="ExternalInput")
    dgow_d = nc.dram_tensor("dgow", [P, TILES], f32, kind="ExternalInput")
    z_d = nc.dram_tensor("z", [pl.NPC_PAD, D], f32, kind="ExternalOutput")

    # batched phase-2 gathers: group tiles while sum(K) <= BATCH_K
    BATCH_K = 64
    batches = []
    b0 = 0
    while b0 < TILES:
        b1 = b0 + 1
        ks = Kt[b0]
        while b1 < TILES and ks + Kt[b1] <= BATCH_K:
            ks += Kt[b1]
            b1 += 1
        batches.append((b0, b1, ks))
        b0 = b1

    with tile.TileContext(nc) as tc:
        with (
            tc.tile_pool(name="consts", bufs=1) as cpool,
            tc.tile_pool(name="own", bufs=3) as p1pool,
            tc.tile_pool(name="gather", bufs=2) as gpool,
            tc.tile_pool(name="work", bufs=2) as wpool,
        ):
            # gathers depend only on idx — load it first so they start at once
            idx_sb = cpool.tile([P, SX], i32)
            nc.sync.dma_start(out=idx_sb[:], in_=idx_d[:, :])
            normsl_sb = cpool.tile([P, SX], f32)
            nc.sync.dma_start(out=normsl_sb[:], in_=normsl_d[:, :])
            wrep_sb = cpool.tile([P, 128], f32)
            nc.sync.dma_start(out=wrep_sb[:], in_=wrep_d[:, :])
            b128_sb = cpool.tile([P, 1], f32)
            nc.sync.dma_start(out=b128_sb[:], in_=b128_d[:, :])

            # ---- own-node scalars (p1b, norm) per tile; overlaps first gathers ----
            p1bT = cpool.tile([P, TILES], f32)
            normow = cpool.tile([P, TILES], f32)
            dgow_sb = cpool.tile([P, TILES], f32)
            nc.sync.dma_start(out=dgow_sb[:], in_=dgow_d[:, :])
            dgclip = cpool.tile([P, TILES], f32)
            nc.vector.tensor_scalar(
                out=dgclip[:], in0=dgow_sb[:], scalar1=1.0, scalar2=None, op0=OP.max
            )
            dgrec = cpool.tile([P, TILES], f32)
            nc.vector.reciprocal(out=dgrec[:], in_=dgclip[:])
            nc.scalar.activation(out=normow[:], in_=dgrec[:], func=AF.Sqrt)

            OB = 8
            for t0 in range(0, TILES, OB):
                tn = min(OB, TILES - t0)
                xo = p1pool.tile([P, OB * 64], f32, tag="xo")
                xov = xo[:].rearrange("p (i f) -> p i f", f=64)
                nc.sync.dma_start(
                    out=xov[:, 0:tn, :],
                    in_=xown_d[:, t0 * 64 : (t0 + tn) * 64].rearrange(
                        "p (t f) -> p t f", f=64
                    ),
                )
                tmpo = p1pool.tile([P, OB * 64], f32, tag="tmpo")
                tov = tmpo[:].rearrange("p (i f) -> p i f", f=64)
                nc.vector.tensor_tensor(
                    out=tov[:, 0:tn, :],
                    in0=xov[:, 0:tn, :],
                    in1=wrep_sb[:, 64:128]
                    .rearrange("p (o f) -> p o f", o=1)
                    .to_broadcast([P, tn, 64]),
                    op=OP.mult,
                )
                redo = wpool.tile([P, OB], f32, tag="redo")
                nc.vector.tensor_reduce(
                    out=redo[:, 0:tn],
                    in_=tov[:, 0:tn, :],
                    axis=mybir.AxisListType.X,
                    op=OP.add,
                )
                nc.vector.tensor_scalar(
                    out=p1bT[:, t0 : t0 + tn],
                    in0=redo[:, 0:tn],
                    scalar1=b128_sb[:, 0:1],
                    scalar2=None,
                    op0=OP.add,
                )

            # ---- phase 2: gather raw x rows + per-edge gate + aggregate ----
            coff = [0]
            for k in Kt:
                coff.append(coff[-1] + k)

            for b0, b1, ks in batches:
                s0 = coff[b0]
                ga = gpool.tile([P, (BATCH_K + 8) * D64], f32, tag="ga")
                gav = ga[:].rearrange("p (k f) -> p k f", f=D64)
                # one [P,1] indirect per slot column (only validated HW shape)
                for col in range(ks):
                    nc.gpsimd.indirect_dma_start(
                        out=ga[:, col * D64 : (col + 1) * D64],
                        out_offset=None,
                        in_=xp_d[:, :],
                        in_offset=bass.IndirectOffsetOnAxis(
                            ap=idx_sb[:, s0 + col : s0 + col + 1], axis=0
                        ),
                    )
                koff = 0
                for t in range(b0, b1):
                    K = Kt[t]
                    xsl = gav[:, koff : koff + K, :]
                    # p2 = x_src . w_src per edge
                    t2 = wpool.tile([P, K * 64], f32, tag="t2")
                    nc.vector.tensor_tensor(
                        out=t2[:].rearrange("p (k f) -> p k f", f=64),
                        in0=xsl,
                        in1=wrep_sb[:, 0:64]
                        .rearrange("p (o f) -> p o f", o=1)
                        .to_broadcast([P, K, 64]),
                        op=OP.mult,
                    )
                    p2d = wpool.tile([P, K], f32, tag="p2d")
                    nc.vector.tensor_reduce(
                        out=p2d[:],
                        in_=t2[:].rearrange("p (k f) -> p k f", f=64),
                        axis=mybir.AxisListType.X,
                        op=OP.add,
                    )
                    tt = wpool.tile([P, K], f32, tag="tt")
                    nc.scalar.activation(
                        out=tt[:], in_=p2d[:], func=AF.Tanh, bias=p1bT[:, t : t + 1]
                    )
                    ee = wpool.tile([P, K], f32, tag="ee")
                    nc.vector.tensor_tensor(
                        out=ee[:],
                        in0=tt[:],
                        in1=normsl_sb[:, s0 + koff : s0 + koff + K],
                        op=OP.mult,
                    )
                    m = wpool.tile([P, K * 64], f32, tag="m")
                    eev = (
                        ee[:]
                        .rearrange("p (k o) -> p k o", o=1)
                        .to_broadcast([P, K, 64])
                    )
                    nc.vector.tensor_tensor(
                        out=m[:].rearrange("p (k f) -> p k f", f=64),
                        in0=xsl,
                        in1=eev,
                        op=OP.mult,
                    )
                    red = wpool.tile([P, 64], f32, tag="red")
                    nc.vector.tensor_reduce(
                        out=red[:],
                        in_=m[:].rearrange("p (k f) -> p f k", f=64),
                        axis=mybir.AxisListType.X,
                        op=OP.add,
                    )
                    zt = wpool.tile([P, 64], f32, tag="zt")
                    nc.vector.tensor_scalar(
                        out=zt[:],
                        in0=red[:],
                        scalar1=normow[:, t : t + 1],
                        scalar2=None,
                        op0=OP.mult,
                    )
                    nc.sync.dma_start(out=z_d[t * P : (t + 1) * P, :], in_=zt[:])
                    koff += K
    nc.compile()
    return nc


_BUILD_CACHE = {}


def build(x, gate_w, gate_b, src, dst, ncores=8):
    pl, in_maps = _prep(x, gate_w, gate_b, src, dst, ncores)
    key = (pl.N, pl.E, pl.ncores, tuple(int(k) for k in pl.Kt))
    nc = _BUILD_CACHE.get(key)
    if nc is None:
        nc = _build_nc(pl)
        _BUILD_CACHE[key] = nc
    return pl, in_maps, nc


def _assemble(pl, outs):
    N, D = pl.N, pl.D
    z = np.zeros((N, D), dtype=np.float32)
    for c in range(pl.ncores):
        nodes = pl.core_nodes[c]
        real = nodes >= 0
        z[nodes[real]] = outs[c][real]
    return z


def kernel(x, gate_w, gate_b, src, dst):
    global LAST_RESULTS
    from concourse.bass_utils import run_bass_kernel_spmd

    pl, in_maps, nc = build(x, gate_w, gate_b, src, dst)
    res = run_bass_kernel_spmd(
        nc,
        in_maps,
        core_ids=list(range(pl.ncores)),
        trace=bool(int(os.environ.get("FAGCN_TRACE", "0"))),
    )
    LAST_RESULTS = res
    outs = [r["z"] for r in res.results]
    return _assemble(pl, outs)
